# revision 3
# baseline (speedup 1.0000x reference)
"""Trainium2 Bass kernel for the BF16Indexer sparse-attention problem.

Computes, for B=1, M=2048, H=32, D=128, N=4096:
    logits = einsum('bmhd,bnd->bmhn', q, k)          (fp32 accum)
    o      = einsum('bmhn,bmh->bmn', relu(logits), w) / sqrt(D)

Sharding: M (query tokens) split across 8 cores; k replicated.

Per-core algorithm (M_loc = 256 rows, mh = M_loc*H = 8192, 64 mh-tiles):

The baseline ran BOTH matmuls through the PE rhs-stream port (~218us of
PE ingest).  This version keeps mm1 on the rhs port but feeds the
head-reduction (mm2) through the PE's *weight* port, which otherwise
sits idle during mm1:

  unit = (group g of 16 mh-tiles, n-window w of 1024 cols, tile t):
  - mm1 (PE):   2x matmul([128,512]) -> psum1 [128=(4m,32h), 1024] fp32
  - drain (ACT or DVE, one fused instr): y' = bf16(relu(scale*psum1))
  - mm2 (PE, B-form): for each 128-col slice s of y':
        matmul(out=psum2[:, 64s+4t:+4], lhsT=y'[:,128s:+128],
               rhs=wblk4[:, 4*tg:+4])
    y' enters as the STATIONARY operand (fast-weight-load, 2 bf16/cyc)
    and the 4-col rhs makes the array cost ~tiny.  The block-column
    rhs routes sum_h w[m,h]*y'[(m,h),n] to output [128=n, 4=m] --
    final o values (transposed), no accumulation chain needed.
  - per (g,w): drain psum2 [128,512] -> SBUF -> one 3D-AP DMA to o^T

PSUM: psum1 = 3 x [128,1024] (6 banks), psum2 = 2 x [128,512] (2 banks).
Loop order is group-outer so qT streams in at ~16GB/s instead of all
within the first window.  Output o is produced TRANSPOSED [N, M_loc];
the host un-transposes (marshalling only).

kernel(**inputs) takes the FULL inputs and returns the FULL
(1, 2048, 4096) fp32 output.
"""

import math
import numpy as np
import ml_dtypes

import concourse.bass as bass
import concourse.mybir as mybir
import concourse.tile as tile
from concourse import bacc
from concourse.bass_utils import run_bass_kernel_spmd

# Problem constants (hardcoded per harness contract)
B, M, H, D, N = 1, 2048, 32, 128, 4096
N_CORES = 8
M_LOC = M // N_CORES              # 256 query rows per core
MH = M_LOC * H                    # 8192
N_TILES = MH // 128               # 64 mh-tiles (4 m's each)
SOFTMAX_SCALE = 1.0 / math.sqrt(float(D))

G_TILES = 16                      # mh-tiles per group
N_GROUPS = N_TILES // G_TILES     # 4
W_COLS = 1024                     # n-cols per window
N_WINDOWS = N // W_COLS           # 4
N_SLICES = W_COLS // 128          # 8 B-chunks per unit


def build_nc():
    nc = bacc.Bacc("TRN2", target_bir_lowering=False, debug=False)

    bf16 = mybir.dt.bfloat16
    f32 = mybir.dt.float32

    qT_d = nc.dram_tensor("qT", [128, MH], bf16, kind="ExternalInput")
    kT_d = nc.dram_tensor("kT", [128, N], bf16, kind="ExternalInput")
    wblk4_d = nc.dram_tensor("wblk4", [128, N_TILES * 4], bf16,
                             kind="ExternalInput")
    # o is stored TRANSPOSED: [n, m_loc]
    oT_d = nc.dram_tensor("oT", [N, M_LOC], f32, kind="ExternalOutput")

    with tile.TileContext(nc) as tc:
        with (
            tc.tile_pool(name="const", bufs=1) as const_pool,
            tc.tile_pool(name="ypool", bufs=4) as ypool,
            tc.tile_pool(name="psum1", bufs=3, space="PSUM") as psum1,
            tc.tile_pool(name="psum2", bufs=2, space="PSUM") as psum2,
            tc.tile_pool(name="ostage", bufs=2) as ostage,
        ):
            qT = const_pool.tile([128, MH], bf16)
            kT = const_pool.tile([128, N], bf16)
            wblk4 = const_pool.tile([128, N_TILES * 4], bf16)

            # --- initial DMAs, chunked so early tiles unblock quickly ---
            nc.sync.dma_start(kT[:, :1024], kT_d[:, :1024])
            nc.scalar.dma_start(wblk4[:], wblk4_d[:])

            # warm the ACT spline tables while DMAs run
            warm = const_pool.tile([128, 1], bf16)
            nc.gpsimd.memset(warm[:], 0)
            nc.scalar.activation(warm[:], warm[:],
                                 mybir.ActivationFunctionType.Relu)

            # warm the PE (HAM un-throttles after ~3.4us of activity)
            wsrc = const_pool.tile([128, 128], bf16)
            nc.gpsimd.memset(wsrc[:], 0)
            wps = psum1.tile([128, 1024], f32, tag="p1", name="warm_ps")
            for _ in range(52):
                nc.tensor.matmul(wps[:, :128], wsrc[:], wsrc[:],
                                 start=True, stop=True)

            def chunked(eng, dst, src, width, edges):
                lo = 0
                for hi in edges:
                    hi = min(hi, width)
                    if hi > lo:
                        eng.dma_start(dst[:, lo:hi], src[:, lo:hi])
                    lo = hi
                if lo < width:
                    eng.dma_start(dst[:, lo:], src[:, lo:])

            # qT group 0 first, then the rest; kT remainder
            chunked(nc.gpsimd, qT, qT_d, MH, [256, 1024, 2048, 4096, 6144])
            nc.sync.dma_start(kT[:, 1024:2048], kT_d[:, 1024:2048])
            nc.sync.dma_start(kT[:, 2048:], kT_d[:, 2048:])

            # --- main pipeline ---
            units = [(g, w, t) for g in range(N_GROUPS)
                     for w in range(N_WINDOWS) for t in range(G_TILES)]
            D1 = 2   # drain trails mm1 by D1 units
            D2 = 4   # B-form mm2 trails mm1 by D2 units

            p1_of = {}   # unit idx -> psum1 tile
            y_of = {}    # unit idx -> y' tile
            p2_of = {}   # (g, w) -> psum2 tile

            def emit_mm1(i):
                g, w, t = units[i]
                tg = g * G_TILES + t
                p1 = psum1.tile([128, 1024], f32, tag="p1",
                                name=f"p1_{i % 3}")
                qT_t = qT[:, bass.ts(tg, 128)]
                for c in range(2):
                    nc.tensor.matmul(
                        p1[:, bass.ts(c, 512)],
                        qT_t,
                        kT[:, bass.ds(w * W_COLS + c * 512, 512)],
                        start=True, stop=True,
                    )
                p1_of[i] = p1

            def emit_drain(j):
                # one fused relu+scale instr over the whole [128,1024] tile
                p1 = p1_of.pop(j)
                y_t = ypool.tile([128, W_COLS], bf16, tag="y",
                                 name=f"y_{j % 4}")
                if j % 11 < 6:
                    nc.scalar.activation(
                        y_t[:], p1[:],
                        mybir.ActivationFunctionType.Relu,
                        scale=SOFTMAX_SCALE,
                    )
                else:
                    nc.vector.tensor_scalar(
                        y_t[:], p1[:], SOFTMAX_SCALE, 0.0,
                        mybir.AluOpType.mult, mybir.AluOpType.max,
                    )
                y_of[j] = y_t

            def emit_mm2(k):
                g, w, t = units[k]
                tg = g * G_TILES + t
                y_t = y_of.pop(k)
                if (g, w) not in p2_of:
                    p2_of[(g, w)] = psum2.tile([128, 512], f32, tag="p2",
                                               name=f"p2_{(g * 4 + w) % 2}")
                p2 = p2_of[(g, w)]
                for s in range(N_SLICES):
                    nc.tensor.matmul(
                        p2[:, bass.ds(64 * s + 4 * t, 4)],
                        y_t[:, bass.ts(s, 128)],
                        wblk4[:, bass.ts(tg, 4)],
                        start=(t == 0 and s == 0),
                        stop=(t == G_TILES - 1 and s == N_SLICES - 1),
                        skip_group_check=True,
                    )
                if t == G_TILES - 1:
                    finish_gw(g, w, p2_of.pop((g, w)))

            def finish_gw(g, w, p2):
                # psum2 -> SBUF (split across both engines) -> 8 2D DMAs
                ost = ostage.tile([128, 512], f32, tag="ost",
                                  name=f"ost_{(g * 4 + w) % 2}")
                nc.scalar.copy(ost[:, :256], p2[:, :256])
                nc.vector.tensor_copy(ost[:, 256:], p2[:, 256:])
                # dst rows n = 1024w + 128s + p, cols m = 64g + 4t + j
                for s in range(N_SLICES):
                    nc.sync.dma_start(
                        oT_d[bass.ds(w * W_COLS + 128 * s, 128),
                             bass.ds(64 * g, 64)],
                        ost[:, bass.ds(64 * s, 64)],
                    )

            n_units = len(units)
            for i in range(n_units):
                emit_mm1(i)
                if i - D1 >= 0:
                    emit_drain(i - D1)
                if i - D2 >= 0:
                    emit_mm2(i - D2)
            for j in range(n_units - D1, n_units):
                emit_drain(j)
            for k in range(n_units - D2, n_units):
                emit_mm2(k)

    nc.compile()
    return nc


def marshal_core_inputs(q, k, weights, core):
    """Host-side layout marshalling for one core (no arithmetic)."""
    bf16 = ml_dtypes.bfloat16

    q_sh = np.asarray(q[0, core * M_LOC:(core + 1) * M_LOC])    # (m_loc, H, D)
    qT = np.ascontiguousarray(q_sh.reshape(MH, D).T)            # (128, mh)
    kT = np.ascontiguousarray(np.asarray(k[0]).T)               # (128, n)

    w_sh = np.asarray(weights[core * M_LOC:(core + 1) * M_LOC, 0, :])  # (m_loc, H)
    # wblk4[32j + h, 4t + j] = w[4t + j, h]
    wblk4 = np.zeros((128, N_TILES * 4), dtype=bf16)
    w_r = w_sh.reshape(N_TILES, 4, H)                           # (t, j, h)
    for j in range(4):
        for h in range(H):
            wblk4[32 * j + h, 4 * np.arange(N_TILES) + j] = w_r[:, j, h]

    return {"qT": qT, "kT": kT, "wblk4": wblk4}


_NC_CACHE = {}


def _get_nc():
    if "nc" not in _NC_CACHE:
        _NC_CACHE["nc"] = build_nc()
    return _NC_CACHE["nc"]


def run(inputs, trace=False):
    nc = _get_nc()
    in_maps = [marshal_core_inputs(inputs["q"], inputs["k"], inputs["weights"], c)
               for c in range(N_CORES)]
    res = run_bass_kernel_spmd(nc, in_maps, list(range(N_CORES)), trace=trace)
    # oT is [N, M_LOC] per core -> transpose and concat along m
    out = np.concatenate(
        [np.ascontiguousarray(res.results[c]["oT"].T) for c in range(N_CORES)],
        axis=0)
    return out[None], res  # (1, M, N) fp32


def kernel(q, k, weights):
    out, _ = run({"q": q, "k": k, "weights": weights})
    return out


# revision 4
# speedup vs baseline: 1.5203x; 1.5203x over previous
"""Trainium2 Bass kernel for the BF16Indexer sparse-attention problem.

Computes, for B=1, M=2048, H=32, D=128, N=4096:
    logits = einsum('bmhd,bnd->bmhn', q, k)          (fp32 accum)
    o      = einsum('bmhn,bmh->bmn', relu(logits), w) / sqrt(D)

Sharding: M (query tokens) split across 8 cores; k replicated.

Per-core algorithm (M_loc = 256 rows, mh = M_loc*H = 8192, 64 mh-tiles):

The baseline ran BOTH matmuls through the PE rhs-stream port (~218us of
PE ingest).  This version keeps mm1 on the rhs port but feeds the
head-reduction (mm2) through the PE's *weight* port, which otherwise
sits idle during mm1:

  unit = (group g of 16 mh-tiles, n-window w of 1024 cols, tile t):
  - mm1 (PE):   2x matmul([128,512]) -> psum1 [128=(4m,32h), 1024] fp32
  - drain (ACT or DVE, one fused instr): y' = bf16(relu(scale*psum1))
  - mm2 (PE, B-form): for each 128-col slice s of y':
        matmul(out=psum2[:, 64s+4t:+4], lhsT=y'[:,128s:+128],
               rhs=wblk4[:, 4*tg:+4])
    y' enters as the STATIONARY operand (fast-weight-load, 2 bf16/cyc)
    and the 4-col rhs makes the array cost ~tiny.  The block-column
    rhs routes sum_h w[m,h]*y'[(m,h),n] to output [128=n, 4=m] --
    final o values (transposed), no accumulation chain needed.
  - per (g,w): drain psum2 [128,512] -> SBUF -> one 3D-AP DMA to o^T

PSUM: psum1 = 3 x [128,1024] (6 banks), psum2 = 2 x [128,512] (2 banks).
Loop order is group-outer so qT streams in at ~16GB/s instead of all
within the first window.  Output o is produced TRANSPOSED [N, M_loc];
the host un-transposes (marshalling only).

kernel(**inputs) takes the FULL inputs and returns the FULL
(1, 2048, 4096) fp32 output.
"""

import math
import numpy as np
import ml_dtypes

import concourse.bass as bass
import concourse.mybir as mybir
import concourse.tile as tile
from concourse import bacc
from concourse.bass_utils import run_bass_kernel_spmd

# Problem constants (hardcoded per harness contract)
B, M, H, D, N = 1, 2048, 32, 128, 4096
N_CORES = 8
M_LOC = M // N_CORES              # 256 query rows per core
MH = M_LOC * H                    # 8192
N_TILES = MH // 128               # 64 mh-tiles (4 m's each)
SOFTMAX_SCALE = 1.0 / math.sqrt(float(D))

G_TILES = 16                      # mh-tiles per group
N_GROUPS = N_TILES // G_TILES     # 4
W_COLS = 1024                     # n-cols per window
N_WINDOWS = N // W_COLS           # 4
N_SLICES = W_COLS // 128          # 8 B-chunks per unit


def build_nc():
    nc = bacc.Bacc("TRN2", target_bir_lowering=False, debug=False)

    bf16 = mybir.dt.bfloat16
    f32 = mybir.dt.float32

    qT_d = nc.dram_tensor("qT", [128, MH], bf16, kind="ExternalInput")
    kT_d = nc.dram_tensor("kT", [128, N], bf16, kind="ExternalInput")
    wblk4_d = nc.dram_tensor("wblk4", [128, N_TILES * 4], bf16,
                             kind="ExternalInput")
    # o is stored TRANSPOSED: [n, m_loc]
    oT_d = nc.dram_tensor("oT", [N, M_LOC], f32, kind="ExternalOutput")

    with tile.TileContext(nc) as tc:
        with (
            tc.tile_pool(name="const", bufs=1) as const_pool,
            tc.tile_pool(name="ypool", bufs=4) as ypool,
            tc.tile_pool(name="psum1", bufs=3, space="PSUM") as psum1,
            tc.tile_pool(name="psum2", bufs=2, space="PSUM") as psum2,
            tc.tile_pool(name="ostage", bufs=2) as ostage,
        ):
            qT = const_pool.tile([128, MH], bf16)
            kT = const_pool.tile([128, N], bf16)
            wblk4 = const_pool.tile([128, N_TILES * 4], bf16)

            # --- initial DMAs, chunked so early tiles unblock quickly ---
            nc.sync.dma_start(kT[:, :1024], kT_d[:, :1024])
            nc.scalar.dma_start(wblk4[:], wblk4_d[:])

            # warm the ACT spline tables while DMAs run
            warm = const_pool.tile([128, 1], bf16)
            nc.gpsimd.memset(warm[:], 0)
            nc.scalar.activation(warm[:], warm[:],
                                 mybir.ActivationFunctionType.Relu)

            # warm the PE (HAM un-throttles after ~3.4us of activity)
            wsrc = const_pool.tile([128, 128], bf16)
            nc.gpsimd.memset(wsrc[:], 0)
            wps = psum1.tile([128, 1024], f32, tag="p1", name="warm_ps")
            for _ in range(52):
                nc.tensor.matmul(wps[:, :128], wsrc[:], wsrc[:],
                                 start=True, stop=True)

            def chunked(eng, dst, src, width, edges):
                lo = 0
                for hi in edges:
                    hi = min(hi, width)
                    if hi > lo:
                        eng.dma_start(dst[:, lo:hi], src[:, lo:hi])
                    lo = hi
                if lo < width:
                    eng.dma_start(dst[:, lo:], src[:, lo:])

            # qT group 0 first, then the rest; kT remainder
            chunked(nc.gpsimd, qT, qT_d, MH, [256, 1024, 2048, 4096, 6144])
            nc.sync.dma_start(kT[:, 1024:2048], kT_d[:, 1024:2048])
            nc.sync.dma_start(kT[:, 2048:], kT_d[:, 2048:])

            # --- main pipeline ---
            units = [(g, w, t) for g in range(N_GROUPS)
                     for w in range(N_WINDOWS) for t in range(G_TILES)]
            D1 = 2   # drain trails mm1 by D1 units
            D2 = 4   # B-form mm2 trails mm1 by D2 units

            p1_of = {}   # unit idx -> psum1 tile
            y_of = {}    # unit idx -> y' tile
            p2_of = {}   # (g, w) -> psum2 tile

            def emit_mm1(i):
                g, w, t = units[i]
                tg = g * G_TILES + t
                p1 = psum1.tile([128, 1024], f32, tag="p1",
                                name=f"p1_{i % 3}")
                qT_t = qT[:, bass.ts(tg, 128)]
                for c in range(2):
                    nc.tensor.matmul(
                        p1[:, bass.ts(c, 512)],
                        qT_t,
                        kT[:, bass.ds(w * W_COLS + c * 512, 512)],
                        start=True, stop=True,
                    )
                p1_of[i] = p1

            def emit_drain(j):
                # one fused relu+scale instr over the whole [128,1024] tile
                p1 = p1_of.pop(j)
                y_t = ypool.tile([128, W_COLS], bf16, tag="y",
                                 name=f"y_{j % 4}")
                if j % 2 == 0:
                    nc.scalar.activation(
                        y_t[:], p1[:],
                        mybir.ActivationFunctionType.Relu,
                        scale=SOFTMAX_SCALE,
                    )
                else:
                    nc.vector.tensor_scalar(
                        y_t[:], p1[:], SOFTMAX_SCALE, 0.0,
                        mybir.AluOpType.mult, mybir.AluOpType.max,
                    )
                y_of[j] = y_t

            def emit_mm2(k):
                g, w, t = units[k]
                tg = g * G_TILES + t
                y_t = y_of.pop(k)
                if (g, w) not in p2_of:
                    p2_of[(g, w)] = psum2.tile([128, 512], f32, tag="p2",
                                               name=f"p2_{(g * 4 + w) % 2}")
                p2 = p2_of[(g, w)]
                for s in range(N_SLICES):
                    nc.tensor.matmul(
                        p2[:, bass.ds(64 * s + 4 * t, 4)],
                        y_t[:, bass.ts(s, 128)],
                        wblk4[:, bass.ts(tg, 4)],
                        start=(t == 0 and s == 0),
                        stop=(t == G_TILES - 1 and s == N_SLICES - 1),
                        skip_group_check=True,
                    )
                if t == G_TILES - 1:
                    finish_gw(g, w, p2_of.pop((g, w)))

            def finish_gw(g, w, p2):
                # psum2 -> SBUF (split across both engines) -> 8 2D DMAs
                ost = ostage.tile([128, 512], f32, tag="ost",
                                  name=f"ost_{(g * 4 + w) % 2}")
                nc.scalar.copy(ost[:, :256], p2[:, :256])
                nc.vector.tensor_copy(ost[:, 256:], p2[:, 256:])
                # dst rows n = 1024w + 128s + p, cols m = 64g + 4t + j
                for s in range(N_SLICES):
                    nc.sync.dma_start(
                        oT_d[bass.ds(w * W_COLS + 128 * s, 128),
                             bass.ds(64 * g, 64)],
                        ost[:, bass.ds(64 * s, 64)],
                    )

            n_units = len(units)
            for i in range(n_units):
                emit_mm1(i)
                if i - D1 >= 0:
                    emit_drain(i - D1)
                if i - D2 >= 0:
                    emit_mm2(i - D2)
            for j in range(n_units - D1, n_units):
                emit_drain(j)
            for k in range(n_units - D2, n_units):
                emit_mm2(k)

    nc.compile()
    return nc


def marshal_core_inputs(q, k, weights, core):
    """Host-side layout marshalling for one core (no arithmetic)."""
    bf16 = ml_dtypes.bfloat16

    q_sh = np.asarray(q[0, core * M_LOC:(core + 1) * M_LOC])    # (m_loc, H, D)
    qT = np.ascontiguousarray(q_sh.reshape(MH, D).T)            # (128, mh)
    kT = np.ascontiguousarray(np.asarray(k[0]).T)               # (128, n)

    w_sh = np.asarray(weights[core * M_LOC:(core + 1) * M_LOC, 0, :])  # (m_loc, H)
    # wblk4[32j + h, 4t + j] = w[4t + j, h]
    wblk4 = np.zeros((128, N_TILES * 4), dtype=bf16)
    w_r = w_sh.reshape(N_TILES, 4, H)                           # (t, j, h)
    for j in range(4):
        for h in range(H):
            wblk4[32 * j + h, 4 * np.arange(N_TILES) + j] = w_r[:, j, h]

    return {"qT": qT, "kT": kT, "wblk4": wblk4}


_NC_CACHE = {}


def _get_nc():
    if "nc" not in _NC_CACHE:
        _NC_CACHE["nc"] = build_nc()
    return _NC_CACHE["nc"]


def run(inputs, trace=False):
    nc = _get_nc()
    in_maps = [marshal_core_inputs(inputs["q"], inputs["k"], inputs["weights"], c)
               for c in range(N_CORES)]
    res = run_bass_kernel_spmd(nc, in_maps, list(range(N_CORES)), trace=trace)
    # oT is [N, M_LOC] per core -> transpose and concat along m
    out = np.concatenate(
        [np.ascontiguousarray(res.results[c]["oT"].T) for c in range(N_CORES)],
        axis=0)
    return out[None], res  # (1, M, N) fp32


def kernel(q, k, weights):
    out, _ = run({"q": q, "k": k, "weights": weights})
    return out
